# revision 2
# baseline (speedup 1.0000x reference)
"""Causal self-attention (B=4, S=2048, D=1024, single head) on 8 TRN2 cores, v2.

Sharding: core c -> batch b = c//2, query-parity h = c%2. Each core computes
full K/V projections for its batch (bf16) and attention for its 8 query
tiles (q-tiles 2s+h, s=0..7, 128 rows each). SPMD: slot s always processes
E_s = 2(s+1) key tiles; a host-built additive mask removes columns past the
true causal extent (parity lives in the data, not the program).

v3 vs v2: K/V projections are pair-split — each core computes only its
own sequence-half of K^T and V (the h-dependence lives in the host-sliced
xh_own input, keeping the program SPMD), exchanges halves with its batch
peer via a DRAM AllGather over replica pairs, and reads back the full K/V.
PE matmul rows drop from ~484k to ~353k per rep.

v2 vs v1: all matmul operands bf16 (half the HBM/DMA bytes at the same PE
row rate), weights+x loaded once per rep (no per-half reload), Q^T kept
SBUF-resident (no DRAM spill), V projected directly in [s,e] layout with
x^T-tile stationary (no V^T transpose pass), softmax max/exp read scores
straight from PSUM (no f32 SBUF staging of scores).

Device layouts (per core, SBUF bf16 unless noted):
  xh  [128, 8*2048]  x[b]^T, d-group g at cols [g*2048:+2048]
  xqs [128, 8*1024]  x[b]^T own q-cols (slot-indexed), d-group g likewise
  w*t [128, 8*1024]  W^T q/k/v, d-group g at cols [g*1024:+1024]
  kt  [128, 8*2048]  K^T, e-group g at cols [g*2048:+2048]
  vv  [128, 16*1024] V, s-tile t at cols [t*1024:+1024]
  qt  [128, 8*1024]  Q^T, e-group g at cols [g*1024:+1024]
  mk  [128, 8*256] f32 additive slot masks
"""
import os
import sys

import numpy as np

for _p in ("/opt/trn_rl_repo", "/root/.axon_site/_ro/trn_rl_repo"):
    if os.path.isdir(_p) and _p not in sys.path:
        sys.path.insert(0, _p)

import concourse.bass as bass
import concourse.mybir as mybir
import concourse.tile as tile
from concourse.bass_utils import run_bass_kernel_spmd

B, S, D = 4, 2048, 1024
P = 128
SCALE = 1.0 / float(np.sqrt(D))
F32 = mybir.dt.float32
BF16 = mybir.dt.bfloat16
NCORES = 8


def _legalize_single_wait(nc):
    """Walrus in this image encodes at most one sync wait per instruction.
    Split each multi-wait instruction into (n-1) prepended same-engine
    NoOps carrying one wait each."""
    for fn in nc.m.functions:
        for block in fn.blocks:
            out = []
            for inst in block.instructions:
                si = inst.sync_info
                if si is not None and len(si.on_wait) > 1:
                    waits = list(si.on_wait)
                    for w in waits[:-1]:
                        out.append(mybir.InstNoOp(
                            name=nc.get_next_instruction_name(),
                            engine=inst.engine,
                            sync_info=mybir.SyncInfo(on_wait=[w],
                                                     on_update=[]),
                            bass_nofuse=True,
                            text_hint="waitsplit",
                        ))
                    inst.sync_info = mybir.SyncInfo(
                        on_wait=[waits[-1]], on_update=list(si.on_update))
                out.append(inst)
            try:
                block.instructions[:] = out
            except TypeError:
                block.instructions = out


def _build_program(reps=1, legalize=True, mode="full"):
    """mode: 'full' replicates the whole body per rep; 'p1'/'p2' replicate
    only the projection/attention phase (for ablation timing)."""
    nc = bass.Bass("TRN2", target_bir_lowering=False, debug=False,
                   num_devices=NCORES)

    xt = nc.dram_tensor("xt", [D, 1024], BF16, kind="ExternalInput").ap()
    xq = nc.dram_tensor("xq", [D, 1024], BF16, kind="ExternalInput").ap()
    wqt = nc.dram_tensor("wqt", [D, D], BF16, kind="ExternalInput").ap()
    wkt = nc.dram_tensor("wkt", [D, D], BF16, kind="ExternalInput").ap()
    wvt = nc.dram_tensor("wvt", [D, D], BF16, kind="ExternalInput").ap()
    mask = nc.dram_tensor("mask", [P, 16 * P], F32, kind="ExternalInput").ap()
    ident = nc.dram_tensor("ident", [P, P], BF16, kind="ExternalInput").ap()
    out = nc.dram_tensor("out", [1024, D], F32, kind="ExternalOutput").ap()

    xt_v = xt.rearrange("(g p) s -> p g s", p=P)     # [128, 8, 1024]
    xq_v = xq.rearrange("(g p) q -> p g q", p=P)     # [128, 8, 1024]
    w_vs = {"q": wqt.rearrange("(g p) e -> p g e", p=P),
            "k": wkt.rearrange("(g p) e -> p g e", p=P),
            "v": wvt.rearrange("(g p) e -> p g e", p=P)}

    with tile.TileContext(nc) as tc:
        from contextlib import ExitStack

        persist = ExitStack()
        const_pool = persist.enter_context(tc.tile_pool(name="cst", bufs=1))
        mk = const_pool.tile([P, 16 * P], F32)   # slot masks
        idn = const_pool.tile([P, P], BF16)      # identity for PE transpose
        nc.sync.dma_start(out=mk[:], in_=mask)
        nc.sync.dma_start(out=idn[:], in_=ident)

        RG = [[0, 1], [2, 3], [4, 5], [6, 7]]   # batch-pair replica groups

        def load_and_p1(rep_st):
            big_pool = rep_st.enter_context(tc.tile_pool(name="big", bufs=1))
            dram_pool = rep_st.enter_context(
                tc.tile_pool(name="xch", bufs=1, space="DRAM"))
            xh = big_pool.tile([P, 8 * 1024], BF16, tag="xh")   # own s-half
            xqs = big_pool.tile([P, 8 * 1024], BF16, tag="xqs")
            wts = {}
            for pj in ("k", "v", "q"):
                wt_ = big_pool.tile([P, 8 * 1024], BF16, tag=f"w{pj}",
                                    name=f"w{pj}")
                nc.sync.dma_start(
                    out=wt_[:].rearrange("p (g e) -> p g e", g=8),
                    in_=w_vs[pj])
                wts[pj] = wt_
            nc.sync.dma_start(
                out=xh[:].rearrange("p (g s) -> p g s", g=8), in_=xt_v)
            nc.sync.dma_start(
                out=xqs[:].rearrange("p (g q) -> p g q", g=8), in_=xq_v)

            kt_lo = big_pool.tile([P, 8 * 1024], BF16, tag="ktl")
            kt_hi = big_pool.tile([P, 8 * 1024], BF16, tag="kth")
            vv_lo = big_pool.tile([P, 8 * 1024], BF16, tag="vvl")
            vv_hi = big_pool.tile([P, 8 * 1024], BF16, tag="vvh")
            qt = big_pool.tile([P, 8 * 1024], BF16, tag="qt")

            stgK = dram_pool.tile([1024, 1024], BF16, name="stgK", tag="stgK")
            stgV = dram_pool.tile([1024, 1024], BF16, name="stgV", tag="stgV")
            agK = dram_pool.tile([2048, 1024], BF16, name="agK", tag="agK")
            agV = dram_pool.tile([2048, 1024], BF16, name="agV", tag="agV")
            stgK_v = stgK[:].rearrange("(g p) s -> p g s", p=P)
            stgV_v = stgV[:].rearrange("(t p) e -> p t e", p=P)

            # =============== phase 1: projections =================
            with ExitStack() as ph1:
                ps1_pool = ph1.enter_context(
                    tc.tile_pool(name="ps1", bufs=4, space="PSUM"))
                ps2_pool = ph1.enter_context(
                    tc.tile_pool(name="ps2", bufs=2, space="PSUM"))
                stg_pool = ph1.enter_context(tc.tile_pool(name="stg", bufs=3))

                # ---- K^T own s-half: e-tile c, stationary wk[g,c] ----
                for c in range(8):
                    pk = [ps1_pool.tile([P, 512], F32, tag="pk",
                                        name=f"pk{c}{j}") for j in range(2)]
                    for g in range(8):
                        for j in range(2):
                            nc.tensor.matmul(
                                pk[j][:],
                                wts["k"][:, g * 1024 + c * P:
                                         g * 1024 + c * P + P],
                                xh[:, g * 1024 + j * 512:
                                   g * 1024 + (j + 1) * 512],
                                start=(g == 0), stop=(g == 7))
                    kst = stg_pool.tile([P, 1024], BF16, tag="kst",
                                        name=f"kst{c}")
                    for j in range(2):
                        nc.scalar.copy(kst[:, j * 512:(j + 1) * 512],
                                       pk[j][:])
                    nc.sync.dma_start(out=stgK_v[:, c, :], in_=kst[:])

                # pair AllGather of K^T halves; rank order fixes layout
                nc.gpsimd.collective_compute(
                    "AllGather", mybir.AluOpType.bypass,
                    replica_groups=RG,
                    ins=[stgK.opt()], outs=[agK.opt()])
                nc.sync.dma_start(
                    out=kt_lo[:].rearrange("p (g s) -> p g s", g=8),
                    in_=agK[0:1024, :].rearrange("(g p) s -> p g s", p=P))
                nc.sync.dma_start(
                    out=kt_hi[:].rearrange("p (g s) -> p g s", g=8),
                    in_=agK[1024:2048, :].rearrange("(g p) s -> p g s", p=P))

                # ---- V own s-half direct: s-tile t, stationary xh[g,t] ----
                for t in range(8):
                    pv = ps2_pool.tile([P, 1024], F32, tag="pv",
                                       name=f"pv{t}")
                    for g in range(8):
                        for eh in range(2):
                            nc.tensor.matmul(
                                pv[:, eh * 512:(eh + 1) * 512],
                                xh[:, g * 1024 + t * P:g * 1024 + t * P + P],
                                wts["v"][:, g * 1024 + eh * 512:
                                         g * 1024 + (eh + 1) * 512],
                                start=(g == 0), stop=(g == 7))
                    vst = stg_pool.tile([P, 1024], BF16, tag="kst",
                                        name=f"vst{t}")
                    nc.scalar.copy(vst[:], pv[:])
                    nc.sync.dma_start(out=stgV_v[:, t, :], in_=vst[:])

                nc.gpsimd.collective_compute(
                    "AllGather", mybir.AluOpType.bypass,
                    replica_groups=RG,
                    ins=[stgV.opt()], outs=[agV.opt()])
                nc.sync.dma_start(
                    out=vv_lo[:].rearrange("p (t e) -> p t e", t=8),
                    in_=agV[0:1024, :].rearrange("(t p) e -> p t e", p=P))
                nc.sync.dma_start(
                    out=vv_hi[:].rearrange("p (t e) -> p t e", t=8),
                    in_=agV[1024:2048, :].rearrange("(t p) e -> p t e", p=P))

                # ---- Q^T: e-tile c, stationary wq[g,c], moving xqs ----
                for c in range(8):
                    pq = ps2_pool.tile([P, 1024], F32, tag="pv",
                                       name=f"pq{c}")
                    for g in range(8):
                        for qh in range(2):
                            nc.tensor.matmul(
                                pq[:, qh * 512:(qh + 1) * 512],
                                wts["q"][:, g * 1024 + c * P:
                                         g * 1024 + c * P + P],
                                xqs[:, g * 1024 + qh * 512:
                                    g * 1024 + (qh + 1) * 512],
                                start=(g == 0), stop=(g == 7))
                    nc.scalar.copy(qt[:, c * 1024:(c + 1) * 1024], pq[:])
            return kt_lo, kt_hi, vv_lo, vv_hi, qt

        def p2(kt_lo, kt_hi, vv_lo, vv_hi, qt):
            # ================= phase 2: attention =================
            with ExitStack() as ph2:
                we_pool = ph2.enter_context(tc.tile_pool(name="wex", bufs=2))
                wt_sb_pool = ph2.enter_context(
                    tc.tile_pool(name="wtsb", bufs=2))
                o_pool = ph2.enter_context(tc.tile_pool(name="osb", bufs=1))
                st_pool = ph2.enter_context(tc.tile_pool(name="stat", bufs=8))
                psc_pool = ph2.enter_context(
                    tc.tile_pool(name="psS", bufs=4, space="PSUM"))
                pso_pool = ph2.enter_context(
                    tc.tile_pool(name="psO", bufs=1, space="PSUM"))
                pst_pool = ph2.enter_context(
                    tc.tile_pool(name="psW", bufs=1, space="PSUM"))

                for s in range(8):
                    E = 2 * (s + 1)          # k-tiles of 128
                    L = E * P                # k-cols: 256..2048
                    nch = (L + 511) // 512
                    mxp = st_pool.tile([P, 4], F32, tag="mx")
                    pss = []
                    for kch in range(nch):
                        w = min(512, L - kch * 512)
                        ps = psc_pool.tile([P, 512], F32, tag="sc",
                                           name=f"sc{s}{kch}")
                        pss.append(ps)
                        ksrc = kt_lo if kch < 2 else kt_hi
                        kloc = (kch % 2) * 512
                        for g in range(8):
                            nc.tensor.matmul(
                                ps[:, :w],
                                qt[:, g * 1024 + s * P:g * 1024 + s * P + P],
                                ksrc[:, g * 1024 + kloc:
                                     g * 1024 + kloc + w],
                                start=(g == 0), stop=(g == 7))
                        if kch == nch - 1:
                            # mask the final 256 cols in-place in PSUM
                            nc.vector.tensor_add(
                                ps[:, w - 256:w], ps[:, w - 256:w],
                                mk[:, s * 256:(s + 1) * 256])
                        nc.vector.reduce_max(mxp[:, kch:kch + 1], ps[:, :w],
                                             axis=mybir.AxisListType.X)

                    negm = st_pool.tile([P, 1], F32, tag="st")
                    nc.vector.reduce_max(negm[:], mxp[:, :nch],
                                         axis=mybir.AxisListType.X,
                                         negate=True)

                    wexp = we_pool.tile([P, 2048], BF16, tag="wex")
                    for kch in range(nch):
                        w = min(512, L - kch * 512)
                        nc.scalar.activation(
                            wexp[:, kch * 512:kch * 512 + w],
                            pss[kch][:, :w],
                            mybir.ActivationFunctionType.Exp,
                            bias=negm[:])

                    ell = st_pool.tile([P, 1], F32, tag="st")
                    nc.vector.reduce_sum(ell[:], wexp[:, :L],
                                         axis=mybir.AxisListType.X)
                    rinv = st_pool.tile([P, 1], F32, tag="st")
                    nc.vector.reciprocal(rinv[:], ell[:])

                    # transpose W (pack 4 tiles per PSUM bank)
                    wt_sb = wt_sb_pool.tile([P, 2048], BF16, tag="wtsb")
                    for bk in range((E + 3) // 4):
                        ntb = min(4, E - 4 * bk)
                        ptw = pst_pool.tile([P, 512], BF16, tag="ptw")
                        for t4 in range(ntb):
                            ki = 4 * bk + t4
                            nc.tensor.transpose(
                                ptw[:, t4 * P:(t4 + 1) * P],
                                wexp[:, ki * P:(ki + 1) * P], idn[:])
                        nc.vector.tensor_copy(
                            wt_sb[:, 4 * bk * P:4 * bk * P + ntb * P],
                            ptw[:, :ntb * P])

                    # PV: k-tile ki lives in V s-tile ki (vv col block ki)
                    po = pso_pool.tile([P, 1024], F32, tag="po")
                    for ki in range(E):
                        vsrc = vv_lo if ki < 8 else vv_hi
                        kv = (ki % 8) * 1024
                        for eh in range(2):
                            nc.tensor.matmul(
                                po[:, eh * 512:(eh + 1) * 512],
                                wt_sb[:, ki * P:(ki + 1) * P],
                                vsrc[:, kv + eh * 512:
                                     kv + (eh + 1) * 512],
                                start=(ki == 0), stop=(ki == E - 1))

                    o_sb = o_pool.tile([P, 1024], F32, tag="osb")
                    nc.vector.tensor_scalar_mul(o_sb[:], po[:], rinv[:])
                    nc.sync.dma_start(out=out[s * P:(s + 1) * P, :],
                                      in_=o_sb[:])

        if mode == "full":
            for _rep in range(reps):
                with ExitStack() as rep_st:
                    tiles = load_and_p1(rep_st)
                    p2(*tiles)
        elif mode == "p1":
            for _rep in range(reps):
                with ExitStack() as rep_st:
                    tiles = load_and_p1(rep_st)
                    if _rep == reps - 1:
                        p2(*tiles)
        elif mode == "p2":
            with ExitStack() as rep_st:
                tiles = load_and_p1(rep_st)
                for _rep in range(reps):
                    p2(*tiles)
        else:
            raise ValueError(mode)

        persist.close()

    if legalize:
        _legalize_single_wait(nc)
    return nc


_NC = {}


def _get_program(reps=1):
    mode = os.environ.get("KMODE", "full")
    key = (reps, mode)
    if key not in _NC:
        _NC[key] = _build_program(reps, mode=mode)
    return _NC[key]


_RUNNER = None


def _get_runner():
    """Build the jitted shard_map runner once and reuse it across calls."""
    global _RUNNER
    if _RUNNER is None:
        import jax
        import jax.numpy as jnp
        from jax.sharding import Mesh, NamedSharding, PartitionSpec
        from jax.experimental.shard_map import shard_map
        from concourse.bass2jax import (_bass_exec_p, install_neuronx_cc_hook,
                                        partition_id_tensor)
        install_neuronx_cc_hook()
        nc = _get_program(1)

        in_names, out_names, out_avals = [], [], []
        for alloc in nc.m.functions[0].allocations:
            if not isinstance(alloc, mybir.MemoryLocationSet):
                continue
            name = alloc.memorylocations[0].name
            pname = (nc.partition_id_tensor.name if nc.partition_id_tensor
                     else None)
            if alloc.kind == "ExternalInput":
                if name != pname:
                    in_names.append(name)
            elif alloc.kind == "ExternalOutput":
                out_names.append(name)
                out_avals.append(jax.core.ShapedArray(
                    tuple(alloc.tensor_shape), mybir.dt.np(alloc.dtype)))
        n_params = len(in_names)
        all_names = list(in_names) + out_names
        pname = nc.partition_id_tensor.name if nc.partition_id_tensor else None
        if pname is not None:
            all_names.append(pname)

        def _body(*args):
            operands = list(args)
            if pname is not None:
                operands.append(partition_id_tensor())
            return tuple(_bass_exec_p.bind(
                *operands, out_avals=tuple(out_avals),
                in_names=tuple(all_names), out_names=tuple(out_names),
                lowering_input_output_aliases=(),
                sim_require_finite=True, sim_require_nnan=True, nc=nc))

        devices = jax.devices()[:NCORES]
        mesh = Mesh(np.asarray(devices), ("core",))
        spec = PartitionSpec("core")
        nin = n_params + len(out_names)
        donate = tuple(range(n_params, n_params + len(out_names)))
        fn = jax.jit(shard_map(_body, mesh=mesh, in_specs=(spec,) * nin,
                               out_specs=(spec,) * len(out_names),
                               check_rep=False),
                     donate_argnums=donate, keep_unused=True)
        sh = NamedSharding(mesh, spec)
        zshapes = [(NCORES * a.shape[0], *a.shape[1:]) for a in out_avals]
        zdtypes = [a.dtype for a in out_avals]
        zf = jax.jit(lambda zs=tuple(zshapes), zd=tuple(zdtypes):
                     tuple(jnp.zeros(sp, d) for sp, d in zip(zs, zd)),
                     out_shardings=(sh,) * len(zshapes))
        _RUNNER = {"fn": fn, "in_names": in_names, "sh": sh, "zf": zf,
                   "prev_outs": None}
    return _RUNNER


def _make_mask(h):
    i = np.arange(P)[:, None]
    j2 = np.arange(256)[None, :]
    blk = np.where(j2 <= h * P + i, 0.0, -1e30).astype(np.float32)
    return np.tile(blk, (1, 8)).copy()


def _bf16(a):
    import ml_dtypes
    return np.asarray(a).astype(ml_dtypes.bfloat16)


def _make_in_maps(x, Wq, Wk, Wv):
    x16 = _bf16(np.asarray(x, dtype=np.float32))
    # 1/sqrt(D) = 2**-5 exactly; folding into Wq is lossless in bf16
    wqt = _bf16(np.asarray(Wq, dtype=np.float32).T * np.float32(SCALE))
    wkt = _bf16(np.asarray(Wk, dtype=np.float32).T)
    wvt = _bf16(np.asarray(Wv, dtype=np.float32).T)
    wqt = np.ascontiguousarray(wqt)
    wkt = np.ascontiguousarray(wkt)
    wvt = np.ascontiguousarray(wvt)
    ident = np.eye(P)
    ident = _bf16(ident)
    masks = [_make_mask(0), _make_mask(1)]

    in_maps = []
    xts = [np.ascontiguousarray(x16[b].T) for b in range(B)]
    for c in range(NCORES):
        b, h = c // 2, c % 2
        xt_full = xts[b]
        # own sequence-half for the pair-split K/V projection
        xt = np.ascontiguousarray(xt_full[:, h * 1024:(h + 1) * 1024])
        own = np.concatenate([np.arange((2 * s + h) * P, (2 * s + h + 1) * P)
                              for s in range(8)])
        xq = np.ascontiguousarray(xt_full[:, own])
        in_maps.append({"xt": xt, "xq": xq, "wqt": wqt, "wkt": wkt,
                        "wvt": wvt, "mask": masks[h], "ident": ident})
    return in_maps


def _gather_full(res_rows):
    """res_rows: [8*1024, 1024] stacked per-core outputs -> [B, S, D]."""
    r = np.asarray(res_rows).reshape(B, 2, 8, P, D)   # [b, h, s, p, d]
    return np.ascontiguousarray(
        r.transpose(0, 2, 1, 3, 4)).reshape(B, S, D)  # q-tile 2s+h


def kernel(x, Wq, Wk, Wv, _trace=False):
    in_maps = _make_in_maps(x, Wq, Wk, Wv)

    if _trace:
        nc = _get_program()
        res = run_bass_kernel_spmd(nc, in_maps, list(range(NCORES)),
                                   trace=True)
        out = np.empty((B, S, D), dtype=np.float32)
        for c in range(NCORES):
            b, h = c // 2, c % 2
            o = res.results[c]["out"]
            for s in range(8):
                out[b, (2 * s + h) * P:(2 * s + h + 1) * P, :] = \
                    o[s * P:(s + 1) * P, :]
        return out, res

    import jax
    r = _get_runner()
    concat_in = [np.concatenate([np.asarray(in_maps[c][n])
                                 for c in range(NCORES)], axis=0)
                 for n in r["in_names"]]
    # donate the previous call's output buffers (fully overwritten by the
    # kernel) to avoid shipping fresh zero buffers over the link each call
    zs = r["prev_outs"]
    if zs is None:
        zs = jax.block_until_ready(r["zf"]())
    outs = jax.block_until_ready(r["fn"](*concat_in, *zs))
    r["prev_outs"] = outs
    return _gather_full(outs[0])


if __name__ == "__main__":
    rng = np.random.default_rng(0)
    xs = rng.standard_normal((B, S, D), dtype=np.float32)
    ws = [rng.standard_normal((D, D), dtype=np.float32) * SCALE
          for _ in range(3)]
    o = kernel(xs, *ws)
    print("kernel ran, out shape", o.shape, "finite:", np.isfinite(o).all())
